# revision 72
# baseline (speedup 1.0000x reference)
"""NeuroSAT message-passing GNN on 8 TRN2 NeuronCores (Bass/Tile).

Sharding: clause dim sharded 8-way (2048 padded clauses/core); literal dim
permuted so core i owns problem i's 500 vars (+12 pads) as 1024 lit rows
(512 pos + 512 neg).  Per round:
  AllGather L_pre halves (fp8; AG h issued right after l_half(h) of the
  previous round so it hides under compute), GEMM1 LC.T = L_pre.T @ B1,
  C-LSTM + C_pre MLP (last layer emitted transposed), GEMM2 halves ->
  fp8 ReduceScatter halves, L-LSTM + L_pre MLP per half.
M (counts) is exact in fp8e4m3.  Most of M is SBUF-resident (B1_RES b1
groups + B2_RES b2 units); the rest streams from HBM through a rotating
pool so the PE is not HBM-starved mid-GEMM.
"""

import numpy as np
import ml_dtypes

import concourse.bass as bass
import concourse.bacc as bacc
import concourse.mybir as mybir
import concourse.tile as tile
from concourse import bass_utils
from concourse.tile_rust import add_dep_helper

F32 = mybir.dt.float32
BF16 = mybir.dt.bfloat16
FP8 = mybir.dt.float8e4
AF = mybir.ActivationFunctionType
DR = mybir.MatmulPerfMode.DoubleRow

N_CORES = 8
DIM = 128
N_ROUNDS = 16
N_VARS = 4000
VPC = 500            # real vars per core (= vars per problem)
VPAD = 512           # padded vars per core
LL = 2 * VPAD        # 1024 lit rows per core
LPAD = N_CORES * LL  # 8192
CC = 2048            # padded clauses per core
CPAD = N_CORES * CC  # 16384
KL = LPAD // 128     # 64 k-tiles over lits
KC = CC // 128       # 16 k-tiles over clauses

# GEMM2 groups: group g computes 512-lit chunks J_SETS[g]; chunk j covers
# local lit rows [512*(j%2)...) of destination core j//2.  Groups 0,1 cover
# all even j (RS half 0 = every core's rows 0:512); groups 2,3 odd j.
J_SETS = [[0, 2, 4, 6], [8, 10, 12, 14], [1, 3, 5, 7], [9, 11, 13, 15]]

# All of M streams through one deep FIFO pool (no SBUF residency): the
# pool holds a full GEMM's worth of 1 MB units, so each GEMM starts fully
# prefetched and runs PE-bound while the DMA ring refills behind it.
B1_STREAM = tuple(range(16))
B2_STREAM = tuple((g, q) for g in range(4) for q in range(4))
# GEMM1 consumption order == stream issue order (FIFO pool): half 0 first.
G1_ORDER = list(range(16))

N_WARM_G1 = 0        # dummy MMs at GEMM1 start (HAM stays warm; see trace)
N_WARM1 = 0          # dummy MMs, L half 0
N_WARM2 = 0          # dummy MMs, L half 1

nbf = ml_dtypes.bfloat16
nf8 = ml_dtypes.float8_e4m3

_CACHE = {}


def _build():
    """Build + compile the SPMD program once (shape-only, no input values)."""
    if "nc" in _CACHE:
        return _CACHE["nc"]

    nc = bacc.Bacc("TRN2", target_bir_lowering=False, debug=False,
                   num_devices=N_CORES, dynamic_dma_scratch_size=2048)

    def din(name, shape, dt):
        return nc.dram_tensor(name, shape, dt, kind="ExternalInput")

    # b1: 16 packed groups of 4 k-tiles; rows ordered [half h, core c, r<512]
    b1 = din("b1", [KL // 4, DIM, 4 * CC], FP8)
    # b2[g]: group g's 16 k-tiles packed 4-per-DMA: [4 groups, 4 qgrp, 128, 4*2048]
    b2 = din("b2", [4, 4, DIM, 4 * 2048], FP8)
    lh0t = din("lh0t", [DIM, LL], BF16)
    ch0t = din("ch0t", [DIM, CC], BF16)
    b3bc_l = din("b3bc_l", [DIM, DIM], BF16)  # Lmsg_b3 bcast along partitions
    b3bc_c = din("b3bc_c", [DIM, DIM], BF16)  # Cmsg_b3 bcast along partitions

    w = {}
    for p in ("lmsg", "cmsg", "lvote"):
        for i in (1, 2, 3):
            shp = [DIM, 1] if (p == "lvote" and i == 3) else [DIM, DIM]
            w[f"{p}_w{i}t"] = din(f"{p}_w{i}t", shp, BF16)
            bshp = [1, 1] if (p == "lvote" and i == 3) else [DIM, 1]
            w[f"{p}_b{i}"] = din(f"{p}_b{i}", bshp, F32)
    # DR-fused LSTM gate weights (fp8): per gate g, [wih_g | whh_g] for C,
    # [wih_cl_g | wih_fl_g] for L, packed as [128, 4 gates * 2 * 128].
    w["cu_wdr"] = din("cu_wdr", [DIM, 8 * DIM], FP8)
    w["lu_wdr"] = din("lu_wdr", [DIM, 8 * DIM], FP8)
    w["lu_whht"] = din("lu_whht", [DIM, 4 * DIM], BF16)
    cu_bias_d = din("cu_bias", [4, DIM], F32)
    lu_bias_d = din("lu_bias", [4, DIM], F32)

    vote_out = nc.dram_tensor("vote", [1, LL], F32, kind="ExternalOutput")

    with tile.TileContext(nc) as tc, \
         tc.tile_pool(name="const", bufs=1) as const, \
         tc.tile_pool(name="sb", bufs=2) as sb, \
         tc.tile_pool(name="ms", bufs=18) as ms, \
         tc.tile_pool(name="ps", bufs=4, space="PSUM") as ps, \
         tc.tile_pool(name="psg", bufs=4, space="PSUM") as psg, \
         tc.tile_pool(name="dram", bufs=2, space="DRAM") as dram:

        # ---- load constants/weights into SBUF
        cw = {}
        for k in w:
            t = const.tile(list(w[k].shape), w[k].dtype, tag=f"cw_{k}")
            nc.sync.dma_start(t[:], w[k].ap())
            cw[k] = t
        for k, dte in (("cu_bias", cu_bias_d), ("lu_bias", lu_bias_d)):
            t = const.tile([DIM, 4], F32, tag=f"cw_{k}")
            nc.sync.dma_start(t[:], dte.ap().rearrange("g p -> p g"))
            cw[k] = t
        zbf = const.tile([DIM, 512], FP8, tag="zbf")
        nc.vector.memset(zbf[:], 0.0)
        b3l = const.tile([DIM, DIM], BF16, tag="b3l")
        nc.sync.dma_start(b3l[:], b3bc_l.ap())
        b3c = const.tile([DIM, DIM], BF16, tag="b3c")
        nc.sync.dma_start(b3c[:], b3bc_c.ap())

        # ---- persistent state (feature-major); loaded before the bulk
        # resident-M DMA so the round-0 head isn't stuck behind 12 MB.
        lht = const.tile([DIM, LL], BF16, tag="lht")
        lct = const.tile([DIM, LL], F32, tag="lct")
        cht = const.tile([DIM, CC], BF16, tag="cht")
        cct = const.tile([DIM, CC], F32, tag="cct")
        nc.sync.dma_start(lht[:], lh0t.ap())
        nc.sync.dma_start(cht[:], ch0t.ap())
        nc.vector.memset(lct[:], 0.0)
        nc.vector.memset(cct[:], 0.0)

        b1res = {}
        b2res = {}

        def sdma2(dst, src):
            """Split a [128, N] transfer across two scalar-engine queues."""
            nc.scalar.dma_start(dst[0:64, :], src[0:64, :])
            nc.scalar.dma_start(dst[64:DIM, :], src[64:DIM, :])

        def warm(n):
            """Disabled: the Tile scheduler spreads dummy matmuls into
            adjacent GEMM streams where their psum-reuse semaphore waits
            break the MM pipeline. HAM stays at 8/8 from real work."""
            assert n == 0

        def issue_streams(r):
            """Issue all of round r's M streaming loads (self-paced by pool)."""
            tiles = {}
            for g in B1_STREAM:
                t = ms.tile([DIM, 4 * CC], FP8, tag="ms", name=f"b1s{g}_{r}")
                nc.sync.dma_start(t[:], b1.ap()[g, :, :])
                tiles[("b1", g)] = t
            for (g, q) in B2_STREAM:
                t = ms.tile([DIM, 4 * 2048], FP8, tag="ms",
                            name=f"b2s{g}{q}_{r}")
                nc.sync.dma_start(t[:], b2.ap()[g, q, :, :])
                tiles[("b2", g, q)] = t
            return tiles

        def mlp12(x, pfx, sl, n, tagsfx=""):
            """Layers 1+2 of a 3-layer MLP on columns sl of x [128, *].
            ReLU+bias on the vector engine (scalar is the busier one)."""
            cur = x
            for li in (1, 2):
                wt = cw[f"{pfx}_w{li}t"]
                bt = cw[f"{pfx}_b{li}"]
                o = sb.tile([DIM, n], BF16, tag=f"mh{li}",
                            name=f"{pfx}_h{li}{tagsfx}")
                for rc in range(n // 512):
                    c0 = rc * 512
                    pt = ps.tile([DIM, 512], F32, tag="ps", name="mlp_ps")
                    src = cur[:, sl.start + c0:sl.start + c0 + 512] if li == 1 \
                        else cur[:, c0:c0 + 512]
                    nc.tensor.matmul(pt[:], wt[:], src, start=True, stop=True)
                    nc.vector.scalar_tensor_tensor(
                        o[:, c0:c0 + 512], pt[:], bt[:, 0:1], zbf[:],
                        op0=mybir.AluOpType.add, op1=mybir.AluOpType.max)
                cur = o
            return cur

        def mlp3_t(h2, pfx, b3t, n, dst, dst_off, tagsfx=""):
            """Last MLP layer emitted transposed: dst[:, dst_off + t*128]
            tiles are [rows, dim_out] fp8 = h2_tile.T @ w3t + b3 (bcast).
            The four [128,128] outputs share one psum bank."""
            wt = cw[f"{pfx}_w3t"]
            pt = ps.tile([DIM, 512], F32, tag="ps", name="l3t")
            for t in range(n // DIM):
                psl = pt[:, t * DIM:(t + 1) * DIM]
                nc.tensor.matmul(psl, h2[:, t * DIM:(t + 1) * DIM], wt[:],
                                 start=True, stop=True)
                sl = slice(dst_off + t * DIM, dst_off + (t + 1) * DIM)
                nc.vector.tensor_add(dst[:, sl], psl, b3t[:])

        def mlp_chunk(x, pfx, sl, n, out_dt=BF16, tagsfx=""):
            """Full 3-layer MLP (vote path only)."""
            cur = x
            for li in (1, 2, 3):
                wt = cw[f"{pfx}_w{li}t"]
                bt = cw[f"{pfx}_b{li}"]
                m = wt.shape[1]
                o = sb.tile([m, n], out_dt if li == 3 else BF16,
                            tag=("vh3" if li == 3 else f"mh{li}"),
                            name=f"{pfx}_h{li}{tagsfx}")
                for rc in range(n // 512):
                    c0 = rc * 512
                    pt = ps.tile([m, 512], F32, tag="ps", name="mlp_ps")
                    src = cur[:, sl.start + c0:sl.start + c0 + 512] if li == 1 \
                        else cur[:, c0:c0 + 512]
                    nc.tensor.matmul(pt[:], wt[:], src, start=True, stop=True)
                    func = AF.Relu if li < 3 else AF.Identity
                    nc.scalar.activation(o[:, c0:c0 + 512], pt[:], func,
                                         bias=bt[:, 0:1])
                cur = o
            return cur

        def lstm_elementwise(gps, bias, c_st, h_st, rc0, n):
            """gps: 4 psum tiles [128, n] (i,f,g,o); updates states [:, rc0:rc0+n]."""
            sl = slice(rc0, rc0 + n)
            sig_i = sb.tile([DIM, n], BF16, tag="lw_si", name="sig_i")
            sig_f = sb.tile([DIM, n], BF16, tag="lw_sf", name="sig_f")
            tng = sb.tile([DIM, n], BF16, tag="lw_tg", name="tng")
            sig_o = sb.tile([DIM, n], BF16, tag="lw_so", name="sig_o")
            nc.scalar.activation(sig_i[:], gps[0][:], AF.Sigmoid, bias=bias[:, 0:1])
            nc.scalar.activation(sig_f[:], gps[1][:], AF.Sigmoid, bias=bias[:, 1:2])
            nc.scalar.activation(tng[:], gps[2][:], AF.Tanh, bias=bias[:, 2:3])
            nc.scalar.activation(sig_o[:], gps[3][:], AF.Sigmoid, bias=bias[:, 3:4])
            t2 = sb.tile([DIM, n], BF16, tag="lw_t2", bufs=1, name="t2")
            nc.vector.tensor_mul(t2[:], sig_i[:], tng[:])
            nc.vector.tensor_mul(c_st[:, sl], sig_f[:], c_st[:, sl])
            nc.vector.tensor_add(c_st[:, sl], c_st[:, sl], t2[:])
            tnc = sb.tile([DIM, n], BF16, tag="lw_tc", name="tnc")
            nc.scalar.activation(tnc[:], c_st[:, sl], AF.Tanh)
            nc.vector.tensor_mul(h_st[:, sl], sig_o[:], tnc[:])

        def gemm1(lpre_sb, streams, n_warm):
            """GEMM1: LC.T [128, 2048] psum accums over 64 packed k-tiles.
            One explicit LDWEIGHTS per k-pair; the 4 column matmuls reuse the
            loaded stationary (ldweights=False). Safe here: no other PE work
            can be scheduled into GEMM1's window (everything later depends on
            its final psum tiles)."""
            lct_ps = [psg.tile([DIM, 512], F32, tag="psg", name=f"g1_{i}")
                      for i in range(4)]
            for gi, grp in enumerate(G1_ORDER):
                src = b1res[grp] if grp in b1res else streams[("b1", grp)]
                b1v = src[:].rearrange("p (t c) -> p t c", c=CC)
                for kk in (0, 2):
                    k = 4 * grp + kk
                    lf = lpre_sb[k // 32]
                    t0 = k % 32
                    lk = lf[:, t0 * DIM:(t0 + 2) * DIM].rearrange(
                        "p (j d) -> p j d", j=2)
                    nc.tensor.ldweights(lk, perf_mode=DR)
                    for c4 in range(4):
                        mm = nc.tensor.matmul(
                            lct_ps[c4][:], lk,
                            b1v[:, kk:kk + 2, c4 * 512:(c4 + 1) * 512],
                            start=(gi == 0 and kk == 0),
                            stop=(gi == 15 and kk == 2),
                            perf_mode=DR)
                        mm.ins.ldweights = False
            return lct_ps

        def c_phase(lct_ps):
            """C-LSTM (DR-fused gates) + C_pre MLP (last layer transposed).
            All [LC | Ch] gate inputs are copied out upfront so the four
            GEMM1 psum banks free immediately and chunks pipeline."""
            cpre_kt = sb.tile([DIM, KC * DIM], FP8, tag="cpre_kt", bufs=1)
            lcchs = []
            for rc in range(4):
                sl = slice(rc * 512, (rc + 1) * 512)
                lcch = sb.tile([DIM, 1024], FP8, tag="lcch", bufs=4,
                               name=f"lcch{rc}")
                if rc % 2 == 0:
                    nc.scalar.activation(lcch[:, 0:512], lct_ps[rc][:],
                                         AF.Identity)
                else:
                    nc.vector.tensor_copy(lcch[:, 0:512], lct_ps[rc][:])
                nc.vector.tensor_copy(lcch[:, 512:1024], cht[:, sl])
                lcchs.append(lcch)
            # PE program order: all 16 gate MMs first (chunk c+1's gates are
            # not blocked behind chunk c's elementwise-dependent MLP), then
            # the per-chunk MLPs, which become ready while later gates run.
            gpss = []
            for rc in range(4):
                lv = lcchs[rc][:].rearrange("p (j n) -> p j n", j=2)
                gps = [ps.tile([DIM, 512], F32, tag="ps", name=f"cg{i}")
                       for i in range(4)]
                for g in range(4):
                    wv = cw["cu_wdr"][:, g * 256:(g + 1) * 256].rearrange(
                        "p (j d) -> p j d", j=2)
                    nc.tensor.matmul(gps[g][:], wv, lv, start=True, stop=True,
                                     perf_mode=DR)
                gpss.append(gps)
                lstm_elementwise(gps, cw["cu_bias"], cct, cht, rc * 512, 512)
            for rc in range(4):
                h2 = mlp12(cht, "cmsg", slice(rc * 512, (rc + 1) * 512), 512,
                           tagsfx="_c")
                mlp3_t(h2, "cmsg", b3c, 512, cpre_kt, rc * 512)
            return cpre_kt

        def gemm2_group(cpre_kt, g, rs_bufs, streams, r):
            """One GEMM2 group: 4 psum accums over KC k-tiles; stage to RS buf."""
            cl_ps = [psg.tile([DIM, 512], F32, tag="psg", name=f"cl{g}_{i}")
                     for i in range(4)]
            for q in range(4):
                key = (g, q)
                src = b2res[key] if key in b2res else streams[("b2", g, q)]
                b2v = src[:].rearrange("p (t c) -> p t c", c=2048)
                for kk in (0, 2):
                    k = 4 * q + kk
                    ck = cpre_kt[:, k * DIM:(k + 2) * DIM].rearrange(
                        "p (j d) -> p j d", j=2)
                    for i in range(4):
                        nc.tensor.matmul(
                            cl_ps[i][:], ck,
                            b2v[:, kk:kk + 2, i * 512:(i + 1) * 512],
                            start=(k == 0), stop=(k == KC - 2),
                            perf_mode=DR)
            for i in range(4):
                j = J_SETS[g][i]
                h, blk = j % 2, j // 2
                cs = sb.tile([DIM, 512], FP8, tag="cl_st", name="cl_st")
                if i % 2 == 0:
                    nc.vector.tensor_copy(cs[:], cl_ps[i][:])
                else:
                    nc.scalar.activation(cs[:], cl_ps[i][:], AF.Identity)
                nc.scalar.dma_start(rs_bufs[h][blk * DIM:(blk + 1) * DIM, 0:512],
                                    cs[:])

        rg = [list(range(N_CORES))]
        last_cc = [None]

        def collective(kind, op, cin, cout):
            cc = nc.gpsimd.collective_compute(kind, op, replica_groups=rg,
                                              ins=[cin.opt()], outs=[cout.opt()])
            # Collectives serialize on one CC stream; encode issue order as a
            # hard dep so the Tile scheduler cannot reorder them (a hoisted
            # AllGather between the two ReduceScatters stalls the L phase).
            if last_cc[0] is not None:
                add_dep_helper(cc.ins, last_cc[0].ins, sync=True,
                               reason="cc stream issue order")
            last_cc[0] = cc

        def lmsg_tail(h, ag_in, tagsfx=""):
            """L_pre MLP for half h (last layer transposed) -> ag_in DRAM."""
            sl = slice(h * 512, (h + 1) * 512)
            h2 = mlp12(lht, "lmsg", sl, 512, tagsfx="_l")
            st = sb.tile([DIM, 4 * DIM], FP8, tag="lp_st", bufs=1,
                         name="lp_st")
            mlp3_t(h2, "lmsg", b3l, 512, st, 0)
            for t in range(4):
                tsl = slice(t * DIM, (t + 1) * DIM)
                nc.scalar.dma_start(ag_in[tsl, :], st[:, tsl])

        def l_half(h, clfl, r, n_warm, ag_in):
            """L-LSTM (DR-fused cl+flip gates) + L_pre MLP for local half h."""
            sl = slice(h * 512, (h + 1) * 512)
            lv = clfl[:].rearrange("p (j n) -> p j n", j=2)
            warm(n_warm)
            gps = [ps.tile([DIM, 512], F32, tag="ps", name=f"lg{h}_{i}")
                   for i in range(4)]
            for g in range(4):
                gsl = slice(g * DIM, (g + 1) * DIM)
                wv = cw["lu_wdr"][:, g * 256:(g + 1) * 256].rearrange(
                    "p (j d) -> p j d", j=2)
                nc.tensor.matmul(gps[g][:], wv, lv, start=True,
                                 stop=False, perf_mode=DR)
                nc.tensor.matmul(gps[g][:], cw["lu_whht"][:, gsl],
                                 lht[:, sl], start=False, stop=True)
            lstm_elementwise(gps, cw["lu_bias"], lct, lht, h * 512, 512)
            lmsg_tail(h, ag_in)

        def load_lpre(ag_outs):
            """Load AG halves as 2 SBUF tiles of 32 k-tiles each (one big
            DMA per half-partition range, not 16 small serial ones)."""
            lpre_sb = []
            for h in range(2):
                lt = sb.tile([DIM, 32 * DIM], FP8, tag="lpf", bufs=2,
                             name=f"lpf{h}")
                s3 = ag_outs[h].rearrange("(t p) d -> p t d", p=DIM)
                d3 = lt[:].rearrange("p (t d) -> p t d", d=DIM)
                nc.scalar.dma_start(d3[0:64], s3[0:64])
                nc.scalar.dma_start(d3[64:DIM], s3[64:DIM])
                lpre_sb.append(lt)
            return lpre_sb

        # ====== round 0 head: L_pre from Lh0 -> ag_in halves + AG issue ======
        ag_ins = []
        for h in range(2):
            ag_in = dram.tile([512, DIM], FP8, tag=f"ag_in{h}",
                              name=f"ag_in{h}_init")
            lmsg_tail(h, ag_in)
            ag_ins.append(ag_in)
        ag_outs = []
        for h in range(2):
            ag_out = dram.tile([4096, DIM], FP8, tag=f"ag_out{h}",
                               name=f"ag_out{h}_init")
            collective("AllGather", mybir.AluOpType.bypass, ag_ins[h], ag_out)
            ag_outs.append(ag_out)

        for r in range(N_ROUNDS):
            streams = issue_streams(r)
            lpre_sb = load_lpre(ag_outs)
            lct_ps = gemm1(lpre_sb, streams, N_WARM_G1)
            cpre_kt = c_phase(lct_ps)

            # flip inputs for both halves must snapshot pre-update Lh; the
            # half-1 flip source is Lh[:, 0:512] which l_half(0) overwrites.
            clfl = []
            for h in range(2):
                t = sb.tile([DIM, 1024], FP8, tag=f"clfl{h}", bufs=1,
                            name=f"clfl{h}_{r}")
                fsl = slice((1 - h) * 512, (2 - h) * 512)
                nc.vector.tensor_copy(t[:, 512:1024], lht[:, fsl])
                clfl.append(t)

            rs_bufs = [dram.tile([N_CORES * DIM, 512], FP8, tag=f"rs_in{h}",
                                 name=f"rs_in{h}_{r}") for h in range(2)]
            gemm2_group(cpre_kt, 0, rs_bufs, streams, r)
            gemm2_group(cpre_kt, 1, rs_bufs, streams, r)
            ro0 = dram.tile([N_CORES * DIM, 512], FP8, tag="rs_out0",
                            name=f"rs_out0_{r}")
            collective("AllToAll", mybir.AluOpType.bypass, rs_bufs[0], ro0)
            gemm2_group(cpre_kt, 2, rs_bufs, streams, r)
            gemm2_group(cpre_kt, 3, rs_bufs, streams, r)

            ro1 = dram.tile([N_CORES * DIM, 512], FP8, tag="rs_out1",
                            name=f"rs_out1_{r}")
            collective("AllToAll", mybir.AluOpType.bypass, rs_bufs[1], ro1)

            def tree_add(ro, dst):
                """Sum the 8 gathered partial blocks of ro into dst [128,512]."""
                pa = sb.tile([DIM, 8 * 512], FP8, tag="pa", bufs=1,
                             name=f"pa_{r}")
                s3 = ro[:].rearrange("(c p) n -> p c n", p=DIM)
                d3 = pa[:].rearrange("p (c n) -> p c n", n=512)
                for q in range(4):
                    nc.scalar.dma_start(d3[q * 32:(q + 1) * 32],
                                        s3[q * 32:(q + 1) * 32])
                us = []
                for b in range(2):
                    t1 = sb.tile([DIM, 512], BF16, tag="pat", bufs=2,
                                 name="pat")
                    nc.vector.tensor_add(t1[:], pa[:, b * 2048:b * 2048 + 512],
                                         pa[:, b * 2048 + 512:b * 2048 + 1024])
                    t2 = sb.tile([DIM, 512], BF16, tag="pat", bufs=2,
                                 name="pat")
                    nc.vector.tensor_add(
                        t2[:], pa[:, b * 2048 + 1024:b * 2048 + 1536],
                        pa[:, b * 2048 + 1536:b * 2048 + 2048])
                    u = sb.tile([DIM, 512], BF16, tag="pau", bufs=2, name="pau")
                    nc.vector.tensor_add(u[:], t1[:], t2[:])
                    us.append(u)
                nc.vector.tensor_add(dst, us[0][:], us[1][:])

            last = r == N_ROUNDS - 1
            ag_ins = [dram.tile([512, DIM], FP8, tag=f"ag_in{h}",
                                name=f"ag_in{h}_{r}") for h in range(2)]
            tree_add(ro0, clfl[0][:, 0:512])
            l_half(0, clfl[0], r, N_WARM1, ag_ins[0])
            if not last:
                ag_out0 = dram.tile([4096, DIM], FP8, tag="ag_out0",
                                    name=f"ag_out0_{r}")
                collective("AllGather", mybir.AluOpType.bypass,
                           ag_ins[0], ag_out0)

            tree_add(ro1, clfl[1][:, 0:512])
            l_half(1, clfl[1], r, N_WARM2, ag_ins[1])
            if not last:
                ag_out1 = dram.tile([4096, DIM], FP8, tag="ag_out1",
                                    name=f"ag_out1_{r}")
                collective("AllGather", mybir.AluOpType.bypass,
                           ag_ins[1], ag_out1)
                ag_outs = [ag_out0, ag_out1]

        # ---- vote MLP on final Lh -> [1, 1024] f32
        vt0 = mlp_chunk(lht, "lvote", slice(0, 512), 512, out_dt=F32,
                        tagsfx="_v")
        vt1 = mlp_chunk(lht, "lvote", slice(512, 1024), 512, out_dt=F32,
                        tagsfx="_v")
        nc.sync.dma_start(vote_out.ap()[:, 0:512], vt0[:])
        nc.sync.dma_start(vote_out.ap()[:, 512:1024], vt1[:])

    nc.compile()
    _CACHE["nc"] = nc
    return nc


def _perm_rows(lits):
    """Map global lit index -> permuted row (core-major, 1024 rows/core)."""
    lits = np.asarray(lits)
    neg = lits >= N_VARS
    v = np.where(neg, lits - N_VARS, lits)
    core = v // VPC
    r = v % VPC
    return core * LL + np.where(neg, VPAD + r, r)


def _b1_row_order():
    """B1 rows: [half h, core c, r] -> permuted row c*1024 + h*512 + r."""
    order = np.empty(LPAD, np.int64)
    n = 0
    for h in range(2):
        for c in range(N_CORES):
            order[n:n + 512] = c * LL + h * 512 + np.arange(512)
            n += 512
    return order


def host_prep(inp):
    f32 = np.float32
    idx = inp["L_unpack_indices"].astype(np.int64)
    rows = _perm_rows(idx[:, 0])
    M = np.zeros((LPAD, CPAD), np.float32)
    np.add.at(M, (rows, idx[:, 1]), 1.0)

    row_order = _b1_row_order()
    b1s, b2s = [], []
    for i in range(N_CORES):
        blk = M[:, i * CC:(i + 1) * CC]          # [8192, 2048] permuted rows
        b1o = blk[row_order]                      # AG-concat row order
        # pack 4 k-tiles per DMA group: [16, 128, 4*2048]
        b1p = b1o.reshape(16, 4, DIM, CC).transpose(0, 2, 1, 3) \
                 .reshape(16, DIM, 4 * CC)
        b1s.append(np.ascontiguousarray(b1p).astype(nf8))
        bT = blk.T                                # [2048 clauses, 8192 lits]
        grp = []
        for g in range(4):
            cols = np.concatenate([np.arange(j * 512, (j + 1) * 512)
                                   for j in J_SETS[g]])
            gb = bT[:, cols]                      # [2048, 2048]
            gp = gb.reshape(4, 4, DIM, 2048).transpose(0, 2, 1, 3) \
                   .reshape(4, DIM, 4 * 2048)
            grp.append(gp)
        b2s.append(np.ascontiguousarray(np.stack(grp)).astype(nf8))

    def bf(x):
        return np.ascontiguousarray(x).astype(nbf)

    l0 = (inp["L_init_w"][:, 0] + inp["L_init_b"]).astype(f32)
    c0 = (inp["C_init_w"][:, 0] + inp["C_init_b"]).astype(f32)
    common = {
        "lh0t": bf(np.repeat(l0[:, None], LL, axis=1)),
        "ch0t": bf(np.repeat(c0[:, None], CC, axis=1)),
        "b3bc_l": bf(np.tile(inp["Lmsg_b3"].astype(f32)[None, :], (DIM, 1))),
        "b3bc_c": bf(np.tile(inp["Cmsg_b3"].astype(f32)[None, :], (DIM, 1))),
        "lu_whht": bf(inp["Lu_whh"].T),
        "cu_bias": (inp["Cu_bih"] + inp["Cu_bhh"]).astype(f32).reshape(4, DIM),
        "lu_bias": (inp["Lu_bih"] + inp["Lu_bhh"]).astype(f32).reshape(4, DIM),
    }
    # DR-fused fp8 gate weights: [wih_g | whh_g] per gate for C,
    # [wih_cl_g | wih_fl_g] per gate for L.
    cu_wih_t = inp["Cu_wih"].T.astype(f32)   # [128, 512]
    cu_whh_t = inp["Cu_whh"].T.astype(f32)
    lu_wih_t = inp["Lu_wih"].T.astype(f32)   # [256, 512]
    cu_wdr = np.empty((DIM, 8 * DIM), f32)
    lu_wdr = np.empty((DIM, 8 * DIM), f32)
    for g in range(4):
        gsl = slice(g * DIM, (g + 1) * DIM)
        cu_wdr[:, g * 256:g * 256 + DIM] = cu_wih_t[:, gsl]
        cu_wdr[:, g * 256 + DIM:(g + 1) * 256] = cu_whh_t[:, gsl]
        lu_wdr[:, g * 256:g * 256 + DIM] = lu_wih_t[:DIM, gsl]
        lu_wdr[:, g * 256 + DIM:(g + 1) * 256] = lu_wih_t[DIM:, gsl]
    common["cu_wdr"] = np.ascontiguousarray(cu_wdr).astype(nf8)
    common["lu_wdr"] = np.ascontiguousarray(lu_wdr).astype(nf8)

    for p, P in (("lmsg", "Lmsg"), ("cmsg", "Cmsg"), ("lvote", "Lvote")):
        for i in (1, 2, 3):
            common[f"{p}_w{i}t"] = bf(inp[f"{P}_w{i}"].T)
            bshape = (1, 1) if (p == "lvote" and i == 3) else (DIM, 1)
            common[f"{p}_b{i}"] = inp[f"{P}_b{i}"].astype(f32).reshape(bshape)
    return [dict(common, b1=b1s[i], b2=b2s[i]) for i in range(N_CORES)]


def kernel(**inputs):
    inp = {k: np.asarray(v) for k, v in inputs.items()}
    in_maps = host_prep(inp)
    nc = _build()
    res = bass_utils.run_bass_kernel_spmd(nc, in_maps,
                                          core_ids=list(range(N_CORES)))
    probs = np.zeros(N_CORES, np.float32)
    for i in range(N_CORES):
        v = res.results[i]["vote"][0]            # [1024]
        s = v[:VPC].astype(np.float64).sum() + \
            v[VPAD:VPAD + VPC].astype(np.float64).sum()
        probs[i] = np.float32(s / (2 * VPC))
    return probs
